# revision 27
# baseline (speedup 1.0000x reference)
"""ConvGNN message-passing kernel for 8x Trainium2 NeuronCores.

Problem (hardcoded):
    batch:         (4, 50000, 64)  f32
    neighborhoods: (50000, 16)     int64, values in [0, 50000] (50000 = zero node)
    kernel:        (16, 64, 64)    f32
    bias:          (1, 1, 64)      f32
    out[b, n, :] = sum_k  x[b, nb[n, k], :] @ W[k]  + bias

Strategy:
  * Host packs the node table to node-major bf16 rows of (B*CIN)=256 units
    (512 B); zero node (id 50000) is a zero row.
  * 8 cores shard the 50000 output nodes (6250 each). Neighborhood indices are
    identical across b, so one gathered row serves all 4 batch elements.
  * dma_gather indices are int16, so each core's 25 node tiles are grouped
    into 5 regions of 5 tiles (20480 gather positions). The host dedups each
    region's referenced rows into a compact per-region sub-table (<=20480
    rows, int16-addressable) and remaps indices; the device then runs a
    SINGLE gather per tile from its region's sub-table — half the HBM
    traffic of a lo/hi split-pass gather, and no merge pass.
  * On-device gather uses nc.gpsimd.dma_gather(transpose=True) from HBM: the
    DMA xbar sprays each row across 128 partitions, landing data directly in
    [(b, c) partition, position] layout = the matmul rhs orientation.
  * PE: per tile of 256 nodes, 2 slabs x 16 k accumulating matmuls with
    block-diag [[Wk,0],[0,Wk]] stationary tiles -> psum[(2b,o) x 256 nodes].
  * Output written as [slab, (2b, o), n] f32; host unpacks to (B, N, COUT).
"""

import numpy as np
import ml_dtypes

import concourse.bacc as bacc
import concourse.bass as bass
import concourse.mybir as mybir
from concourse.bass_utils import run_bass_kernel_spmd
from concourse.library_config import mlp

# ---------------------------------------------------------------- constants
B, N, K, CIN, COUT = 4, 50000, 16, 64, 64
NCORES = 8
NODES_PER_CORE = N // NCORES          # 6250
NODE_TILE = 256                       # nodes per tile
POS_TILE = NODE_TILE * K              # 4096 gather positions per tile
NT = (NODES_PER_CORE + NODE_TILE - 1) // NODE_TILE   # 25 tiles per core
NPAD = NT * NODE_TILE                 # 6400 padded nodes per core

REG_TILES = 5                         # tiles per gather region
NREG = NT // REG_TILES                # 5 regions per core
REG_ROWS = REG_TILES * POS_TILE       # 20480: max distinct rows per region

ELEM = B * CIN                        # 256 bf16 units per row (512 B)
SLABS = ELEM // 128                   # 2

F32 = mybir.dt.float32
BF16 = mybir.dt.bfloat16
I16 = mybir.dt.int16

BF = ml_dtypes.bfloat16


# ---------------------------------------------------------------- program
def build_program():
    nc = bacc.Bacc("TRN2")

    table = nc.declare_dram_parameter("table", [NREG * REG_ROWS, ELEM], BF16, isOutput=False)
    idx_d = nc.declare_dram_parameter(
        "idx", [NREG, 128, REG_TILES * NODE_TILE], I16, isOutput=False
    )
    w_d = nc.declare_dram_parameter("w", [128, K * 128], BF16, isOutput=False)
    bias_d = nc.declare_dram_parameter("biasx", [128, 1], F32, isOutput=False)
    out_d = nc.declare_dram_parameter("out", [SLABS, 128, NPAD], BF16, isOutput=True)

    from contextlib import ExitStack
    with ExitStack() as ctx:
        e = ctx.enter_context
        idx_sb = e(nc.sbuf_tensor("idx_sb", [128, NT * NODE_TILE], I16))
        w_sb = e(nc.sbuf_tensor("w_sb", [128, K * 128], BF16))
        bias_sb = e(nc.sbuf_tensor("bias_sb", [128, 1], F32))
        NBUF = 4
        g = [
            e(nc.sbuf_tensor(f"g{j}", [128, SLABS, POS_TILE], BF16))
            for j in range(NBUF)
        ]
        stage0 = e(nc.sbuf_tensor("stage0", [128, SLABS * NODE_TILE], BF16))
        stage1 = e(nc.sbuf_tensor("stage1", [128, SLABS * NODE_TILE], BF16))
        ps00 = e(nc.psum_tensor("ps00", [128, NODE_TILE], F32))
        ps01 = e(nc.psum_tensor("ps01", [128, NODE_TILE], F32))
        ps10 = e(nc.psum_tensor("ps10", [128, NODE_TILE], F32))
        ps11 = e(nc.psum_tensor("ps11", [128, NODE_TILE], F32))
        isem = [e(nc.semaphore(f"isem{j}")) for j in range(NREG)]
        wsem = e(nc.semaphore("wsem"))
        gsem = [e(nc.semaphore(f"gsem{j}")) for j in range(NBUF)]
        mm_sem = e(nc.semaphore("mm_sem"))
        evac_sem = e(nc.semaphore("evac_sem"))
        osem = [e(nc.semaphore(f"osem{j}")) for j in range(2)]
        block = e(nc.Block())
        stage = [stage0, stage1]
        psum = [[ps00, ps01], [ps10, ps11]]

        RCOL = REG_TILES * NODE_TILE

        @block.sync
        def _(sync):
            for rg in range(NREG):
                sync.dma_start(
                    out=idx_sb[:, rg * RCOL:(rg + 1) * RCOL],
                    in_=idx_d[rg, :, :],
                ).then_inc(isem[rg], 16)
            sync.dma_start(out=w_sb[:, :], in_=w_d[:, :]).then_inc(wsem, 16)
            sync.dma_start(out=bias_sb[:, :], in_=bias_d[:, :]).then_inc(wsem, 16)
            for i in range(NT):
                buf = i % 2
                sync.wait_ge(evac_sem, i + 1)
                for s in range(SLABS):
                    sync.dma_start(
                        out=out_d[s, :, i * NODE_TILE:(i + 1) * NODE_TILE],
                        in_=stage[buf][:, s * NODE_TILE:(s + 1) * NODE_TILE],
                    ).then_inc(osem[buf], 16)
            for j in range(2):
                sync.wait_ge(osem[j], ((NT - j + 1) // 2) * SLABS * 16)

        @block.gpsimd
        def _(gpsimd):
            gpsimd.load_library(mlp)
            nreg = gpsimd.to_reg(POS_TILE)
            for i in range(NT):
                buf = i % NBUF
                r = i // REG_TILES
                if i % REG_TILES == 0:
                    gpsimd.wait_ge(isem[r], 16)
                if i >= NBUF:
                    gpsimd.wait_ge(mm_sem, i - NBUF + 1)
                sl = slice(i * NODE_TILE, (i + 1) * NODE_TILE)
                gpsimd.dma_gather(
                    out_ap=g[buf][:, :, :],
                    in_ap=table[r * REG_ROWS:(r + 1) * REG_ROWS, :],
                    idxs_ap=idx_sb[:, sl],
                    num_idxs=POS_TILE,
                    num_idxs_reg=nreg,
                    elem_size=ELEM,
                    elem_step=ELEM,
                    transpose=True,
                    single_packet=False,
                ).then_inc(gsem[buf], 16)

        @block.vector
        def _(vector):
            def evac(j):
                jb = j % 2
                vector.wait_ge(mm_sem, j + 1)
                if j >= 2:
                    vector.wait_ge(osem[jb], SLABS * 16 * ((j - 2) // 2 + 1))
                for s in range(SLABS):
                    ins = vector.tensor_add(
                        stage[jb][:, s * NODE_TILE:(s + 1) * NODE_TILE],
                        psum[jb][s][:, :],
                        bias_sb[:, :].to_broadcast([128, NODE_TILE]),
                    )
                    if s == SLABS - 1:
                        ins.then_inc(evac_sem, 1)

            for j in range(NT):
                evac(j)

        @block.tensor
        def _(tensor):
            tensor.wait_ge(wsem, 32)
            for i in range(NT):
                buf = i % NBUF
                pbuf = i % 2
                tensor.wait_ge(gsem[buf], 16 * (i // NBUF + 1))
                if i >= 2:
                    tensor.wait_ge(evac_sem, i - 1)
                for s in range(SLABS):
                    for k in range(K):
                        ins = tensor.matmul(
                            psum[pbuf][s][:, :],
                            w_sb[:, k * 128:(k + 1) * 128],
                            g[buf][:, s, k * NODE_TILE:(k + 1) * NODE_TILE],
                            start=(k == 0),
                            stop=(k == K - 1),
                        )
                if True:
                    ins.then_inc(mm_sem, 1)

    nc.compile()
    return nc


# ---------------------------------------------------------------- host side
def _pack_inputs(batch, neighborhoods, kernel, bias):
    batch = np.asarray(batch, dtype=np.float32)
    nb = np.asarray(neighborhoods, dtype=np.int64)
    w = np.asarray(kernel, dtype=np.float32)
    bias = np.asarray(bias, dtype=np.float32).reshape(COUT)

    # node content: rows 0..N-1 nodes, row N zeros (the ref zero node)
    content = np.zeros((N + 1, ELEM), dtype=BF)
    content[:N] = batch.transpose(1, 0, 2).reshape(N, ELEM).astype(BF)

    # block-diag stationary weight tiles [128, K*128]
    wt = np.zeros((128, K, 128), dtype=BF)
    wbf = w.astype(BF)
    for k in range(K):
        wt[0:64, k, 0:64] = wbf[k]
        wt[64:128, k, 64:128] = wbf[k]
    wt = wt.reshape(128, K * 128)

    bias_t = np.tile(bias, 2).reshape(128, 1).astype(np.float32)

    ids = nb.astype(np.int32)                 # [N, K] in [0, N]
    idx_maps = []
    tables = []
    for c in range(NCORES):
        n0 = c * NODES_PER_CORE
        r = np.full((NPAD, K), N, dtype=np.int32)   # pad nodes -> zero row
        r[:NODES_PER_CORE] = ids[n0:n0 + NODES_PER_CORE]
        # k-major position order per tile: p = k*NODE_TILE + nl
        pos = (
            r.reshape(NT, NODE_TILE, K).transpose(0, 2, 1).reshape(NT, POS_TILE)
        )
        # per-region dedup + remap to compact sub-table ids
        tab = np.zeros((NREG * REG_ROWS, ELEM), dtype=BF)
        idx16 = np.empty((NT, POS_TILE), dtype=np.int16)
        for rg in range(NREG):
            t0 = rg * REG_TILES
            flat = pos[t0:t0 + REG_TILES].reshape(-1)
            uniq, inv = np.unique(flat, return_inverse=True)
            assert len(uniq) <= REG_ROWS
            idx16[t0:t0 + REG_TILES] = (
                inv.astype(np.int16).reshape(REG_TILES, POS_TILE)
            )
            tab[rg * REG_ROWS: rg * REG_ROWS + len(uniq)] = content[uniq]
        tables.append(tab)

        # wrap into [128, NT*NODE_TILE] SBUF layout (16-part wrap, x8 replica),
        # then regroup region-major for contiguous per-region DMA chunks
        a = idx16.reshape(NT, POS_TILE // 16, 16)
        a = a.transpose(0, 2, 1)                   # [t, 16, P/16]
        a = np.tile(a, (1, 8, 1))                  # replicate to 128 parts
        a = a.transpose(1, 0, 2).reshape(128, NT * POS_TILE // 16)
        rcol = REG_TILES * NODE_TILE
        idx_maps.append(
            np.ascontiguousarray(
                a.reshape(128, NREG, rcol).transpose(1, 0, 2)
            )
        )

    return tables, wt, bias_t, idx_maps


_PROGRAM_CACHE = {}


def _run(batch, neighborhoods, kernel, bias, **spmd_kwargs):
    tables, wt, bias_t, idx_maps = _pack_inputs(
        batch, neighborhoods, kernel, bias
    )

    if "nc" not in _PROGRAM_CACHE:
        _PROGRAM_CACHE["nc"] = build_program()
    nc = _PROGRAM_CACHE["nc"]

    in_maps = []
    for c in range(NCORES):
        in_maps.append({
            "table": tables[c],
            "idx": idx_maps[c],
            "w": wt,
            "biasx": bias_t,
        })

    kres = run_bass_kernel_spmd(nc, in_maps, list(range(NCORES)), **spmd_kwargs)
    res = kres.results

    out = np.empty((B, N, COUT), dtype=np.float32)
    for c in range(NCORES):
        o = np.asarray(res[c]["out"])[:, :, :NODES_PER_CORE]   # [S, 128, n]
        o = o.astype(np.float32)
        o = o.reshape(SLABS, 2, COUT, NODES_PER_CORE)          # [s, b01, o, n]
        o = o.transpose(0, 1, 3, 2).reshape(B, NODES_PER_CORE, COUT)
        out[:, c * NODES_PER_CORE:(c + 1) * NODES_PER_CORE, :] = o
    return out, kres


def kernel(batch, neighborhoods, kernel, bias):
    out, _ = _run(batch, neighborhoods, kernel, bias)
    return out
